# revision 14
# baseline (speedup 1.0000x reference)
"""Trainium2 Bass kernel for SimCLR NT-Xent contrastive loss (N=4096, D=512, T=0.5).

Math: with z = rownorm(concat(emb_i, emb_j)) (8192x512) and S = z @ z.T:
  loss = (1/2N) * [ sum_r log(rowsum_r(exp(S/T)) - exp(1/T)) - (1/T) * sum_r S[r, (r+N) mod 2N] ]

Distribution: data-parallel over rows of z. Each of the 8 cores receives a
block-rotated copy of the concatenated input (rotation by 1024*c rows) in TWO
layouts: row-major [8192, 512] bf16 (for row norms + positive-pair dots) and
transposed [512, 8192] bf16 (the matmul operand layout). The same program
computes row block [0:1024) of its rotated similarity matrix against all 8192
columns. Rotation preserves the row set and the +N pair structure (mod 2N).

Per-core pipeline (column-group-outer, 4 groups of 2048, software-pipelined
2 deep so the PE never starves):
  1. load raw bf16 rows; squares w/ fused row-sum (DVE)
  2. rsqrt of norms via Quake seed + 2 Newton steps (DVE); inv*16 -> bf16
  3. inv -> column-ordered DRAM scratch; DMA partition-broadcast -> [128,2048]
  4. load transposed tiles, scale by inv[col] into fp8 (x16) with the
     k-interleaved DoubleRow layout (DVE + GpSimd split)
  5. 32 DoubleRow fp8 matmuls (8 m x 4 n x 2 kk) -> PSUM f32;
     ACT exp(x/128) with fused row-sum accumulation
  positives: raw-row dots (DVE STT, fused accum) * inv[i] * inv[i+4096]
Host merges 8 partial [128,8] tensors (denominator row-sums, pair-dots):
  loss = (sum log(den - e^2) - 2*sum pos) / 8192.
"""

import numpy as np
import ml_dtypes

for _p in ("/opt/trn_rl_repo", "/root/.axon_site/_ro/trn_rl_repo"):
    try:
        import concourse  # noqa: F401
        break
    except ImportError:
        import sys
        if _p not in sys.path:
            sys.path.insert(0, _p)

import concourse.bass as bass
import concourse.bacc as bacc
import concourse.tile as tile
from concourse import mybir
from concourse.bass_utils import run_bass_kernel_spmd

F32 = mybir.dt.float32
I32 = mybir.dt.int32
BF16 = mybir.dt.bfloat16
FP8 = mybir.dt.float8e4
ALU = mybir.AluOpType
AF = mybir.ActivationFunctionType

N_CORES = 8
BATCH = 4096
DIM = 512
ROWS = 2 * BATCH            # 8192
BLOCK = ROWS // N_CORES     # 1024 rows per core
P = 128                     # partitions
KC = DIM // P               # 4 k-chunks
MT = BLOCK // P             # 8 m-tiles
NCG = 4                     # column groups
CGW = ROWS // NCG           # 2048 cols per group
NW = 512                    # matmul free width
TEMP_SCALE = 2.0            # 1/T
SCALE_UP = 16.0             # fp8 pre-scale; exp scale folds 1/SCALE_UP^2
MAGIC = 0x5F3759DF
NG = 8                      # row-load groups (1024 rows each)
GR = ROWS // NG             # 1024
GT = GR // P                # 8 tiles per group


def _build_program():
    nc = bacc.Bacc(trn_type="TRN2")
    xr_in = nc.declare_dram_parameter("xr", [ROWS, DIM], BF16, isOutput=False)
    xt_in = nc.declare_dram_parameter("xt", [DIM, ROWS], BF16, isOutput=False)
    den_out = nc.declare_dram_parameter("den", [P, MT], F32, isOutput=True)
    pos_out = nc.declare_dram_parameter("pos", [P, MT], F32, isOutput=True)

    with tile.TileContext(nc) as tc:
        with tc.tile_pool(name="xg", bufs=3) as xg_pool, \
             tc.tile_pool(name="sq", bufs=3) as sq_pool, \
             tc.tile_pool(name="small", bufs=2) as small_pool, \
             tc.tile_pool(name="ztr", bufs=12) as ztr_pool, \
             tc.tile_pool(name="bcast", bufs=2) as bcast_pool, \
             tc.tile_pool(name="single", bufs=1) as singles, \
             tc.tile_pool(name="escr", bufs=3) as e_pool, \
             tc.tile_pool(name="invd", bufs=2, space="DRAM") as invd_pool, \
             tc.tile_pool(name="mmps", bufs=2, space="PSUM") as mm_psum:

            n2 = singles.tile([P, NG * GT], F32, tag="n2")
            inv = singles.tile([P, NG * GT], F32, tag="inv")
            magic = singles.tile([P, 2 * GT], I32, tag="magic")
            nc.vector.memset(magic, MAGIC)
            accm = singles.tile([P, MT * NCG], F32, tag="accm")
            posraw = singles.tile([P, MT], F32, tag="posraw")
            pos_t = singles.tile([P, MT], F32, tag="pos_t")
            den_t = singles.tile([P, MT], F32, tag="den_t")
            # pinned rows 0-1023 (positive-pair partners for rows 4096-5119)
            xg_pin = singles.tile([P, GT, DIM], BF16, tag="xgpin")
            # scaled fp8 zT tiles, k-interleaved for DoubleRow:
            # zt8[kk][cg][p, i, c] = SCALE_UP * z[col c, d=256*kk+128*i+p]
            zt8 = [[singles.tile([P, 2, CGW], FP8, tag=f"zt8_{kk}_{c}",
                                 name=f"zt8_{kk}_{c}")
                    for c in range(NCG)] for kk in range(2)]

            def prep(cg):
                # --- load raw rows, square + per-tile row-sum into n2 ---
                for h in range(2):
                    g = 2 * cg + h
                    r0 = g * GR
                    xg = xg_pin if g == 0 else xg_pool.tile(
                        [P, GT, DIM], BF16, tag="xg")
                    nc.sync.dma_start(
                        out=xg,
                        in_=xr_in[r0:r0 + GR, :].rearrange(
                            "(a p) d -> p a d", p=P))
                    for a in range(GT):
                        sq = sq_pool.tile([P, DIM], BF16, tag="sq")
                        nc.vector.scalar_tensor_tensor(
                            out=sq, in0=xg[:, a, :], scalar=0.0,
                            in1=xg[:, a, :], op0=ALU.bypass, op1=ALU.mult,
                            accum_out=n2[:, g * GT + a: g * GT + a + 1])
                    if g == 4:
                        # positive pairs: rotated rows [0:1024) x [4096:5120)
                        for a in range(GT):
                            psc = sq_pool.tile([P, DIM], BF16, tag="sq")
                            nc.vector.scalar_tensor_tensor(
                                out=psc, in0=xg_pin[:, a, :], scalar=0.0,
                                in1=xg[:, a, :], op0=ALU.bypass, op1=ALU.mult,
                                accum_out=posraw[:, a: a + 1])
                # --- rsqrt on this cg's 16 norms: Quake seed + 2 Newton ---
                T = 2 * GT
                sl = n2[:, cg * T:(cg + 1) * T]
                isl = inv[:, cg * T:(cg + 1) * T]
                sh = small_pool.tile([P, T], I32, tag="sh")
                nc.vector.tensor_scalar(
                    out=sh, in0=sl.bitcast(I32), scalar1=1, scalar2=None,
                    op0=ALU.logical_shift_right)
                seed = small_pool.tile([P, T], I32, tag="seed")
                nc.vector.scalar_tensor_tensor(
                    out=seed, in0=magic, scalar=0.0, in1=sh,
                    op0=ALU.bypass, op1=ALU.subtract)
                y = seed.bitcast(F32)
                for it in range(2):
                    ta = small_pool.tile([P, T], F32, tag="ta")
                    tb = small_pool.tile([P, T], F32, tag="tb")
                    nc.vector.tensor_mul(out=ta, in0=y, in1=y)
                    nc.vector.scalar_tensor_tensor(
                        out=tb, in0=ta, scalar=-0.5, in1=sl,
                        op0=ALU.mult, op1=ALU.mult)
                    nc.vector.tensor_scalar(
                        out=tb, in0=tb, scalar1=1.5, scalar2=None, op0=ALU.add)
                    dst = isl if it == 1 else y
                    nc.vector.tensor_mul(out=dst, in0=y, in1=tb)
                # inv * SCALE_UP as bf16 for the fp8 pre-scale
                iv16 = small_pool.tile([P, 32], BF16, tag="iv16")
                nc.vector.tensor_scalar(
                    out=iv16[:, 0:T], in0=isl, scalar1=SCALE_UP, scalar2=None,
                    op0=ALU.mult)
                # --- inv -> row-ordered DRAM via DVE 32x32 block transpose,
                # then one contiguous partition-broadcast read back ---
                # iv16[p, m] is row 128m+p; transposed block (b) holds, on
                # partition 32b+i (i<T), rows 128i+32b+j at free offset j.
                ivt = small_pool.tile([P, 32], BF16, tag="ivt")
                nc.vector.transpose(out=ivt, in_=iv16)
                invd = invd_pool.tile([CGW], BF16, tag="invd")
                for b in range(4):
                    nc.scalar.dma_start(
                        out=invd.rearrange("(i b j) -> i b j", b=4, j=32)
                                [:, b, :],
                        in_=ivt[32 * b:32 * b + T, :])
                bc = bcast_pool.tile([P, CGW], BF16, tag="bc")
                nc.scalar.dma_start(
                    out=bc,
                    in_=invd.rearrange("(a f) -> a f", a=1)
                            .partition_broadcast(P))
                # --- load transposed tiles, scale into fp8 DoubleRow layout ---
                for k in range(KC):
                    ztr = ztr_pool.tile([P, CGW], BF16, tag="ztr")
                    nc.sync.dma_start(
                        out=ztr,
                        in_=xt_in[k * P:(k + 1) * P,
                                  cg * CGW:(cg + 1) * CGW])
                    nc.vector.tensor_mul(
                        out=zt8[k // 2][cg][:, k % 2, :], in0=ztr, in1=bc)

            def mmblock(cg):
                for m in range(MT):
                    ps = mm_psum.tile([P, CGW], F32, tag="ps")
                    for n in range(CGW // NW):
                        for kk in range(2):
                            nc.tensor.matmul(
                                ps[:, n * NW:(n + 1) * NW],
                                lhsT=zt8[kk][0][:, :, m * P:(m + 1) * P],
                                rhs=zt8[kk][cg][:, :, n * NW:(n + 1) * NW],
                                start=(kk == 0), stop=(kk == 1),
                                perf_mode=mybir.MatmulPerfMode.DoubleRow)
                    e_scr = e_pool.tile([P, CGW], BF16, tag="escr")
                    nc.scalar.activation(
                        out=e_scr, in_=ps, func=AF.Exp,
                        scale=TEMP_SCALE / (SCALE_UP * SCALE_UP),
                        accum_out=accm[:, m * NCG + cg: m * NCG + cg + 1])

            prep(0)
            prep(1)
            prep(2)
            for cg in range(NCG):
                mmblock(cg)
                if cg + 3 < NCG:
                    prep(cg + 3)

            # --- outputs: raw denominator row-sums + scaled pair-dots ---
            for m in range(MT):
                nc.vector.reduce_sum(
                    out=den_t[:, m:m + 1], in_=accm[:, m * NCG:(m + 1) * NCG],
                    axis=mybir.AxisListType.X)
            nc.vector.tensor_mul(out=pos_t, in0=posraw, in1=inv[:, 0:MT])
            nc.vector.tensor_mul(
                out=pos_t, in0=pos_t, in1=inv[:, 4 * MT:5 * MT])
            nc.sync.dma_start(out=den_out[:, :], in_=den_t)
            nc.sync.dma_start(out=pos_out[:, :], in_=pos_t)

    nc.finalize()
    return nc


_CACHE = {}


def _run(full: np.ndarray, trace: bool = False, **kwargs):
    """Run the SPMD program on all 8 cores; returns BassKernelResults."""
    if "nc" not in _CACHE:
        _CACHE["nc"] = _build_program()
    nc = _CACHE["nc"]
    xbf = full.astype(ml_dtypes.bfloat16)
    in_maps = []
    for c in range(N_CORES):
        xc = np.ascontiguousarray(np.roll(xbf, -BLOCK * c, axis=0))
        in_maps.append({
            "xr": xc,
            "xt": np.ascontiguousarray(xc.T),
        })
    return run_bass_kernel_spmd(
        nc, in_maps, core_ids=list(range(N_CORES)), trace=trace, **kwargs)


def _merge(results) -> np.ndarray:
    logd_sum = 0.0
    pos_sum = 0.0
    e2 = float(np.exp(2.0))
    for r in results:
        den = r["den"].astype(np.float64)
        logd_sum += np.log(den - e2).sum()
        pos_sum += r["pos"].astype(np.float64).sum()
    loss = (logd_sum - TEMP_SCALE * pos_sum) / (2.0 * BATCH)
    return np.array(loss, dtype=np.float32)


def kernel(emb_i: np.ndarray, emb_j: np.ndarray) -> np.ndarray:
    full = np.concatenate(
        [np.asarray(emb_i, np.float32), np.asarray(emb_j, np.float32)], axis=0)
    return _merge(_run(full).results)
